# revision 1
# baseline (speedup 1.0000x reference)
import os

import numpy as np

B, V, P, E = 2, 16, 40320, 16
NCORES = 8
PC = P // NCORES
NPT = B * V * PC
PART = 128
FREE = NPT // PART
FP = FREE // 3
CHUNKS = [(0, FP // 2), (FP // 2, FP // 2), (FP, FP), (2 * FP, FP)]
N_TILES = len(CHUNKS)
COLS_PER_TILE = 17
NCOL = N_TILES * COLS_PER_TILE + 1

_CACHE = {}
LAST_EXEC_NS = None


def _gps_offsets():
    v = os.environ.get("KERNEL_CRPS_GPS", "")
    return frozenset(int(x) for x in v.split(",") if x.strip())


def _stt_offsets():
    v = os.environ.get("KERNEL_CRPS_STT", "9,10,11,12,13,14,15")
    return frozenset(int(x) for x in v.split(",") if x.strip())


def _build_nc(f16: bool, gps=frozenset(), stt=frozenset(range(9, 16))):
    import concourse.bacc as bacc
    from concourse import mybir, tile
    from concourse.mybir import AluOpType

    cdt = mybir.dt.float16 if f16 else mybir.dt.float32
    f32 = mybir.dt.float32

    nc = bacc.Bacc(
        "TRN2",
        target_bir_lowering=False,
        debug=False,
        enable_asserts=False,
        num_devices=NCORES,
    )
    y = nc.dram_tensor("y", [PART, FREE * E], f32, kind="ExternalInput")
    g = nc.dram_tensor("g", [PART, FREE], f32, kind="ExternalInput")
    t = nc.dram_tensor("t", [PART, FREE], f32, kind="ExternalInput")
    out = nc.dram_tensor("acc", [PART, NCOL], f32, kind="ExternalOutput")

    with tile.TileContext(nc) as tc:
        with (
            tc.tile_pool(name="y_pool", bufs=2) as y_pool,
            tc.tile_pool(name="yh_pool", bufs=2) as yh_pool,
            tc.tile_pool(name="sc_pool", bufs=6) as sc_pool,
            tc.tile_pool(name="fix", bufs=1) as fix,
        ):
            gt = fix.tile([PART, FREE], f32)
            tt = fix.tile([PART, FREE], f32)
            th = fix.tile([PART, FREE], cdt)
            acc = fix.tile([PART, NCOL], f32)
            nc.vector.memset(acc[:], 0.0)
            nc.sync.dma_start(out=gt[:], in_=g.ap())
            nc.sync.dma_start(out=tt[:], in_=t.ap())
            nc.vector.scalar_tensor_tensor(
                out=th[:], in0=tt[:], scalar=0.0, in1=gt[:],
                op0=AluOpType.bypass, op1=AluOpType.mult,
                accum_out=acc[:, NCOL - 1:NCOL])

            for j, (off, fp) in enumerate(CHUNKS):
                base = j * COLS_PER_TILE
                yt = y_pool.tile([PART, fp * E], f32)
                yh = yh_pool.tile([PART, fp * E], cdt)
                nc.sync.dma_start(
                    out=yt[:], in_=y.ap()[:, off * E:(off + fp) * E])
                yt_v = yt[:].rearrange("p (f e) -> p f e", e=E)
                yh_w = yh[:].rearrange("p (e f) -> p f e", f=fp)
                g_b = (gt[:, off:off + fp]
                       .unsqueeze(2).broadcast_to([PART, fp, E]))
                nc.vector.scalar_tensor_tensor(
                    out=yh_w, in0=yt_v, scalar=0.0, in1=g_b,
                    op0=AluOpType.bypass, op1=AluOpType.mult,
                    accum_out=acc[:, base + 16:base + 17])
                yh_v = yh[:].rearrange("p (e f) -> p e f", e=E)
                th_b = (th[:, off:off + fp]
                        .unsqueeze(1).broadcast_to([PART, E, fp]))
                mt = sc_pool.tile([PART, E * fp], cdt, tag="sc")
                mt_v = mt[:].rearrange("p (e f) -> p e f", e=E)
                nc.vector.tensor_tensor(
                    mt_v, yh_v, th_b, AluOpType.max)
                nc.scalar.activation(
                    out=mt[:], in_=mt[:],
                    func=mybir.ActivationFunctionType.Copy,
                    accum_out=acc[:, base + 15:base + 16])
                for d in range(1, E):
                    w = (E - d) * fp
                    if d in stt:
                        sc = sc_pool.tile([PART, E * fp], cdt, tag="sc")
                        sc_v = sc[:].rearrange("p (e f) -> p e f", e=E)
                        eng = nc.gpsimd if d in gps else nc.vector
                        eng.scalar_tensor_tensor(
                            out=sc_v[:, 0:E - d, :],
                            in0=yh_v[:, 0:E - d, :], scalar=0.0,
                            in1=yh_v[:, d:E, :],
                            op0=AluOpType.bypass, op1=AluOpType.max,
                            accum_out=acc[:, base + d - 1:base + d])
                    else:
                        pt = sc_pool.tile([PART, E * fp], cdt, tag="sc")
                        pt_v = pt[:].rearrange("p (e f) -> p e f", e=E)
                        nc.vector.tensor_tensor(
                            pt_v[:, 0:E - d, :],
                            yh_v[:, 0:E - d, :], yh_v[:, d:E, :],
                            AluOpType.max)
                        nc.scalar.activation(
                            out=pt[:, 0:w], in_=pt[:, 0:w],
                            func=mybir.ActivationFunctionType.Copy,
                            accum_out=acc[:, base + d - 1:base + d])

            nc.sync.dma_start(out=out.ap(), in_=acc[:])
    nc.compile()
    return nc


def kernel(y_pred, y_target, weights, scale):
    global LAST_EXEC_NS
    from concourse.bass_utils import run_bass_kernel_spmd

    f16 = os.environ.get("KERNEL_CRPS_F32", "0") != "1"
    gps = _gps_offsets()
    stt = _stt_offsets()
    key = ("nc", f16, gps, stt)
    if key not in _CACHE:
        _CACHE[key] = _build_nc(f16, gps, stt)
    nc = _CACHE[key]

    y_pred = np.ascontiguousarray(np.asarray(y_pred, dtype=np.float32))
    y_target = np.ascontiguousarray(np.asarray(y_target, dtype=np.float32))
    weights = np.asarray(weights, dtype=np.float32)
    scale = np.asarray(scale, dtype=np.float32)

    ghat = scale[None, :, None] * weights[None, None, :]
    ghat = np.broadcast_to(ghat, (B, V, P))

    in_maps = []
    for c in range(NCORES):
        sl = slice(c * PC, (c + 1) * PC)
        yc = y_pred[:, :, sl, :].reshape(PART, FREE * E)
        tc_ = y_target[:, :, sl].reshape(PART, FREE)
        gc = ghat[:, :, sl].reshape(PART, FREE)
        in_maps.append({
            "y": np.ascontiguousarray(yc),
            "t": np.ascontiguousarray(tc_),
            "g": np.ascontiguousarray(gc),
        })

    res = run_bass_kernel_spmd(
        nc, in_maps, core_ids=list(range(NCORES)), trace=False)
    LAST_EXEC_NS = res.exec_time_ns

    A_pair = A_mae = S = T1 = 0.0
    for c in range(NCORES):
        a = res.results[c]["acc"].astype(np.float64)
        for j in range(N_TILES):
            base = j * COLS_PER_TILE
            A_pair += a[:, base:base + 15].sum()
            A_mae += a[:, base + 15].sum()
            S += a[:, base + 16].sum()
        T1 += a[:, NCOL - 1].sum()

    MAE_total = (2.0 * A_mae - E * T1 - S) / E
    PAIR_total = (-1.0 / (E * E)) * (2.0 * A_pair - (E - 1) * S)
    npoints = np.asarray(weights, dtype=np.float64).sum()
    result = (MAE_total + PAIR_total) / (npoints * B)
    return np.float32(result)



# revision 3
# speedup vs baseline: 1.0065x; 1.0065x over previous
import os

import numpy as np

B, V, P, E = 2, 16, 40320, 16
NCORES = 8
PC = P // NCORES
NPT = B * V * PC
PART = 128
FREE = NPT // PART

NCH = 6
CHW = FREE // NCH
CTILES = [(0, 420), (420, 420), (840, 420)]

COL_S0 = 0
COL_T1 = COL_S0 + NCH
COL_MAE0 = COL_T1 + 1
COL_PAIR0 = COL_MAE0 + len(CTILES)
NACC = COL_PAIR0 + 48

PE_W = 512

_CACHE = {}
LAST_EXEC_NS = None


def _act_ds():
    v = os.environ.get("KERNEL_CRPS_ACT_D", "")
    out = set()
    for item in v.split(","):
        item = item.strip()
        if item:
            t, d = item.split(":")
            out.add((int(t), int(d)))
    return frozenset(out)


def _build_nc(act_ds=frozenset()):
    import concourse.bacc as bacc
    from concourse import mybir, tile
    from concourse.mybir import AluOpType

    f16 = mybir.dt.float16
    f32 = mybir.dt.float32

    nc = bacc.Bacc(
        "TRN2",
        target_bir_lowering=False,
        debug=False,
        enable_asserts=False,
        num_devices=NCORES,
    )
    y = nc.dram_tensor("y", [PART, FREE * E], f32, kind="ExternalInput")
    g = nc.dram_tensor("g", [PART, FREE], f32, kind="ExternalInput")
    t = nc.dram_tensor("t", [PART, FREE], f32, kind="ExternalInput")
    ones = nc.dram_tensor("ones", [PART, 1], f16, kind="ExternalInput")
    acc_d = nc.dram_tensor("acc", [PART, NACC], f32, kind="ExternalOutput")
    pe_d = nc.dram_tensor("pe", [1, PE_W], f32, kind="ExternalOutput")

    with tile.TileContext(nc) as tc:
        with (
            tc.tile_pool(name="fix", bufs=1) as fix,
            tc.tile_pool(name="ych", bufs=2) as ych,
            tc.tile_pool(name="sc", bufs=3) as scp,
            tc.tile_pool(name="psum", bufs=1, space="PSUM") as psp,
        ):
            gt = fix.tile([PART, FREE], f32)
            tt = fix.tile([PART, FREE], f32)
            th = fix.tile([PART, FREE], f16)
            onest = fix.tile([PART, 1], f16)
            yh = fix.tile([PART, FREE * E], f16)
            acc = fix.tile([PART, NACC], f32)
            pe_stage = fix.tile([1, PE_W], f32)
            pe_row = psp.tile([1, PE_W], f32)

            nc.vector.memset(acc[:], 0.0)
            nc.sync.dma_start(out=gt[:], in_=g.ap())
            nc.sync.dma_start(out=tt[:], in_=t.ap())
            nc.sync.dma_start(out=onest[:], in_=ones.ap())

            nc.vector.scalar_tensor_tensor(
                out=th[:], in0=tt[:], scalar=0.0, in1=gt[:],
                op0=AluOpType.bypass, op1=AluOpType.mult,
                accum_out=acc[:, COL_T1:COL_T1 + 1])

            mm_state = {"n": 0}
            total_mms = {"n": 0}

            def pe_reduce(src_ap, width):
                for off in range(0, width, PE_W):
                    w = min(PE_W, width - off)
                    first = mm_state["n"] == 0
                    mm_state["n"] += 1
                    last = mm_state["n"] == total_mms["n"]
                    nc.tensor.matmul(
                        pe_row[0:1, 0:w],
                        onest[:, 0:1],
                        src_ap[:, off:off + w],
                        start=first,
                        stop=last,
                    )

            n_mm = 0
            for (t0, fw) in CTILES:
                for d in range(1, E):
                    if (CTILES.index((t0, fw)), d) in act_ds:
                        continue
                    wd = (E - d) * fw
                    n_mm += (wd + PE_W - 1) // PE_W
            total_mms["n"] = n_mm

            def emit_chunk(c):
                yt = ych.tile([PART, CHW * E], f32, tag="ych")
                nc.sync.dma_start(
                    out=yt[:], in_=y.ap()[:, c * CHW * E:(c + 1) * CHW * E])
                yt3 = yt[:].rearrange("p (f e) -> p f e", e=E)
                yh3 = (yh[:, c * CHW * E:(c + 1) * CHW * E]
                       .rearrange("p (f e) -> p f e", e=E))
                g3 = (gt[:, c * CHW:(c + 1) * CHW]
                      .unsqueeze(2).broadcast_to([PART, CHW, E]))
                nc.vector.scalar_tensor_tensor(
                    out=yh3, in0=yt3, scalar=0.0, in1=g3,
                    op0=AluOpType.bypass, op1=AluOpType.mult,
                    accum_out=acc[:, COL_S0 + c:COL_S0 + c + 1])

            def emit_tile(ti):
                t0, fw = CTILES[ti]
                yh3 = (yh[:, t0 * E:(t0 + fw) * E]
                       .rearrange("p (f e) -> p f e", e=E))
                mt = scp.tile([PART, fw * E], f16, tag="sc")
                mt3 = mt[:].rearrange("p (f e) -> p f e", e=E)
                th3 = (th[:, t0:t0 + fw]
                       .unsqueeze(2).broadcast_to([PART, fw, E]))
                nc.vector.scalar_tensor_tensor(
                    out=mt3, in0=yh3, scalar=0.0, in1=th3,
                    op0=AluOpType.bypass, op1=AluOpType.max,
                    accum_out=acc[:, COL_MAE0 + ti:COL_MAE0 + ti + 1])
                pair_col = COL_PAIR0 + ti * 15
                for d in range(1, E):
                    wd = (E - d) * fw
                    sc = scp.tile([PART, fw * E], f16, tag="sc")
                    sc3 = sc[:, 0:wd].rearrange("p (f e) -> p f e", e=E - d)
                    nc.vector.tensor_tensor(
                        sc3, yh3[:, :, 0:E - d], yh3[:, :, d:E],
                        AluOpType.max)
                    if (ti, d) in act_ds:
                        nc.scalar.activation(
                            out=sc[:, 0:wd], in_=sc[:, 0:wd],
                            func=mybir.ActivationFunctionType.Copy,
                            accum_out=acc[:, pair_col + d - 1:pair_col + d])
                    else:
                        pe_reduce(sc[:], wd)

            for ti in range(len(CTILES)):
                emit_chunk(2 * ti)
                emit_chunk(2 * ti + 1)
                emit_tile(ti)

            nc.scalar.activation(
                out=pe_stage[:], in_=pe_row[:],
                func=mybir.ActivationFunctionType.Copy)
            nc.sync.dma_start(out=pe_d.ap(), in_=pe_stage[:])
            nc.sync.dma_start(out=acc_d.ap(), in_=acc[:])
    nc.compile()
    return nc


def kernel(y_pred, y_target, weights, scale):
    global LAST_EXEC_NS
    from concourse.bass_utils import run_bass_kernel_spmd

    act_ds = _act_ds()
    key = ("nc", act_ds)
    if key not in _CACHE:
        _CACHE[key] = _build_nc(act_ds)
    nc = _CACHE[key]

    y_pred = np.ascontiguousarray(np.asarray(y_pred, dtype=np.float32))
    y_target = np.ascontiguousarray(np.asarray(y_target, dtype=np.float32))
    weights = np.asarray(weights, dtype=np.float32)
    scale = np.asarray(scale, dtype=np.float32)

    ghat = scale[None, :, None] * weights[None, None, :]
    ghat = np.broadcast_to(ghat, (B, V, P))
    ones = np.ones((PART, 1), dtype=np.float16)

    in_maps = []
    for c in range(NCORES):
        sl = slice(c * PC, (c + 1) * PC)
        yc = y_pred[:, :, sl, :].reshape(PART, FREE * E)
        tc_ = y_target[:, :, sl].reshape(PART, FREE)
        gc = ghat[:, :, sl].reshape(PART, FREE)
        in_maps.append({
            "y": np.ascontiguousarray(yc),
            "t": np.ascontiguousarray(tc_),
            "g": np.ascontiguousarray(gc),
            "ones": ones,
        })

    res = run_bass_kernel_spmd(
        nc, in_maps, core_ids=list(range(NCORES)), trace=False)
    LAST_EXEC_NS = res.exec_time_ns

    S = T1 = A_mae = A_pair = 0.0
    for c in range(NCORES):
        a = res.results[c]["acc"].astype(np.float64)
        p = res.results[c]["pe"].astype(np.float64)
        S += a[:, COL_S0:COL_S0 + NCH].sum()
        T1 += a[:, COL_T1].sum()
        A_mae += a[:, COL_MAE0:COL_MAE0 + len(CTILES)].sum()
        A_pair += a[:, COL_PAIR0:].sum() + p.sum()

    MAE_total = (2.0 * A_mae - E * T1 - S) / E
    PAIR_total = (-1.0 / (E * E)) * (2.0 * A_pair - (E - 1) * S)
    npoints = np.asarray(weights, dtype=np.float64).sum()
    result = (MAE_total + PAIR_total) / (npoints * B)
    return np.float32(result)
